# revision 60
# baseline (speedup 1.0000x reference)
"""Multi-head attention forward (B=2, T=2048, C=1024, 16 heads of dim 64)
sharded 8-way tensor-parallel over heads across 8 TRN2 NeuronCores.

Each core computes 2 heads end-to-end:
  qkv^T = w_c^T @ x^T           (weight-stationary, produces transposed layout)
  S^T_h = k_h @ q_h^T           (head-dim contraction; the two heads' K=64
                                 matmuls use disjoint partition ranges so the
                                 PE packs them into one full-rate pass)
  P^T_h = exp(S^T_h)            (no max subtraction: scores are ~N(0,1), |S|<9)
  y^T_h = [v_h | 1]^T @ P^T_h   (ones column yields softmax denominators)
  out_c = sum_h (y_h/denom) @ w_proj[head rows]   (partial projection, bf16)
Host gathers: out = sum_c out_c  (the tensor-parallel all-reduce).

Pipeline layout: ScalarE's exp stream (the second-busiest resource after the
PE) starts as soon as chunk 0 of batch 0 is projected; in each main slot the
next chunk's score production is interleaved at 2-ktile grain with this
chunk's pv / batch-1 qkv / deferred projections so the in-order PE queue is
never parked behind the exp stream's PSUM WAR dependency. Per-head Ln/Exp
reciprocals are emitted immediately after each head's pv to hide their
latency, and the last chunk's pv+norm are hoisted into the prior slot so the
drain tail is just its projection.
"""

import numpy as np
import ml_dtypes
from contextlib import ExitStack

import concourse.bass as bass
import concourse.bacc as bacc
import concourse.mybir as mybir
import concourse.tile as tile
from concourse.bass_utils import run_bass_kernel_spmd
from concourse.masks import make_identity

F32 = mybir.dt.float32
BF16 = mybir.dt.bfloat16
AFT = mybir.ActivationFunctionType

P = 128
NB = 2        # batches
TB = 2048     # tokens per batch
NT = NB * TB  # 4096 tokens total
C = 1024
KC = C // P   # 8 contraction tiles for the qkv projection
QCH = 512     # q-token chunk
NQC = TB // QCH   # 4 q chunks per batch
NKT = TB // P     # 16 k tiles per batch
NCH = NB * NQC    # 8 chunks total
N_CORES = 8
HEAD_DIM = 64
HD1 = HEAD_DIM + 1


def _build_program(nc: bass.Bass):
    xT = nc.declare_dram_parameter("xT", [C, NT], BF16, isOutput=False)[:]
    # wqkv arrives pre-packed partition-major so its single DMA is 128
    # contiguous 6KB descriptors instead of 1024 x 768B
    wqkv = nc.declare_dram_parameter("wqkv", [P, KC, 384], BF16, isOutput=False)[:]
    wproj = nc.declare_dram_parameter("wproj", [2, HEAD_DIM, C], BF16, isOutput=False)[:]
    out = nc.declare_dram_parameter("out", [NT, C], BF16, isOutput=True)[:]

    with tile.TileContext(nc) as tc, ExitStack() as ctx:
        singles = ctx.enter_context(tc.tile_pool(name="singles", bufs=1))
        xin = ctx.enter_context(tc.tile_pool(name="xin", bufs=16))
        vtp = ctx.enter_context(tc.tile_pool(name="vtp", bufs=2))
        ppool = ctx.enter_context(tc.tile_pool(name="ppool", bufs=3))
        small = ctx.enter_context(tc.tile_pool(name="small", bufs=3))
        ybp = ctx.enter_context(tc.tile_pool(name="ybp", bufs=6))
        opool = ctx.enter_context(tc.tile_pool(name="opool", bufs=6))
        psA = ctx.enter_context(tc.tile_pool(name="psA", bufs=2, space="PSUM"))
        pyP = ctx.enter_context(tc.tile_pool(name="pyP", bufs=2, space="PSUM"))
        psB = ctx.enter_context(tc.tile_pool(name="psB", bufs=2, space="PSUM"))

        # ---------------- constants / persistent tensors ----------------
        # k columns (m=1) first: the first qkv chain is the k-chain, so it
        # can start as soon as this slice + the first x tiles land
        w_sb = singles.tile([P, KC, 384], BF16, tag="w_sb")
        for m in (1, 0, 2):
            nc.sync.dma_start(
                out=w_sb[:, :, m * P : (m + 1) * P],
                in_=wqkv[:, :, m * P : (m + 1) * P],
            )

        wp_sb = singles.tile([P, C], BF16, tag="wp")

        ident = singles.tile([P, P], BF16, tag="ident")

        q_sb = singles.tile([P, NT], BF16, tag="q_sb")
        # k_both rows 0:64 = head0 k dims, rows 64:128 = head1 k dims
        k_both = singles.tile([P, NT], BF16, tag="k_both")
        # v_aug[:, i, h, :] = [v_h for token tile i (64 cols) | ones col]
        v_aug = singles.tile([P, NT // P, 2, HD1], BF16, tag="v_aug")
        nc.vector.memset(v_aug[:, :, :, HEAD_DIM:HD1], 1.0)

        # ---------------- building blocks ----------------
        def xt_load(t, engines=None):
            # engines: spread the DMA triggers across idle engine DGE rings
            # (prologue only) instead of serializing on the sync queue
            tsl = slice(t * QCH, (t + 1) * QCH)
            tiles = []
            for kc in range(KC):
                xt = xin.tile([P, QCH], BF16, tag="xin", name="xt")
                eng = engines[kc % len(engines)] if engines else nc.sync
                eng.dma_start(out=xt[:], in_=xT[kc * P : (kc + 1) * P, tsl])
                tiles.append(xt)
            return tiles

        def qkv_chain(t, xts, m, dest):
            # one 128-row slice (m=0: q both heads, m=1: k both heads)
            tsl = slice(t * QCH, (t + 1) * QCH)
            ps = psB.tile([P, QCH], F32, tag="psB", name="ps")
            for kc in range(KC):
                nc.tensor.matmul(
                    ps[:],
                    lhsT=w_sb[:, kc, m * P : (m + 1) * P],
                    rhs=xts[kc][:],
                    start=(kc == 0),
                    stop=(kc == KC - 1),
                )
            nc.vector.tensor_copy(out=dest[:, tsl], in_=ps[:])

        def qkv_v_chain(t, xts):
            ps = psB.tile([P, QCH], F32, tag="psB", name="ps")
            for kc in range(KC):
                nc.tensor.matmul(
                    ps[:],
                    lhsT=w_sb[:, kc, 2 * P : 3 * P],
                    rhs=xts[kc][:],
                    start=(kc == 0),
                    stop=(kc == KC - 1),
                )
            vt = vtp.tile([P, QCH], BF16, tag="vt")
            nc.vector.tensor_copy(out=vt[:], in_=ps[:])
            return vt

        def v_transpose(t, vt):
            pt = psB.tile([P, QCH], BF16, tag="psB", name="pt")
            for j in range(QCH // P):
                nc.tensor.transpose(
                    pt[:, j * P : (j + 1) * P], vt[:, j * P : (j + 1) * P], ident[:]
                )
            for j in range(QCH // P):
                i = t * (QCH // P) + j
                nc.vector.tensor_copy(
                    out=v_aug[:, i, 0, 0:HEAD_DIM], in_=pt[:, j * P : j * P + HEAD_DIM]
                )
                nc.vector.tensor_copy(
                    out=v_aug[:, i, 1, 0:HEAD_DIM],
                    in_=pt[:, j * P + HEAD_DIM : (j + 1) * P],
                )

        def sc_kt(b, qc, kt, pT):
            # both heads' scores in one pass: disjoint partition ranges let
            # the PE pack the two K=64 matmuls
            qsl = slice(b * TB + qc * QCH, b * TB + (qc + 1) * QCH)
            ksl = slice(b * TB + kt * P, b * TB + (kt + 1) * P)
            ps = psA.tile([P, 2 * QCH], F32, tag="psA", name="ps")
            nc.tensor.matmul(
                ps[:, 0:QCH],
                lhsT=k_both[0:HEAD_DIM, ksl],
                rhs=q_sb[0:HEAD_DIM, qsl],
                start=True,
                stop=True,
            )
            nc.tensor.matmul(
                ps[:, QCH : 2 * QCH],
                lhsT=k_both[HEAD_DIM:P, ksl],
                rhs=q_sb[HEAD_DIM:P, qsl],
                start=True,
                stop=True,
            )
            nc.scalar.activation(out=pT[:, kt, :, :], in_=ps[:], func=AFT.Exp)

        def pv_part(py, b, h, pT, k0, k1):
            for kt in range(k0, k1):
                nc.tensor.matmul(
                    py[0:HD1, :],
                    lhsT=v_aug[:, b * NKT + kt, h, :],
                    rhs=pT[:, kt, h, :],
                    start=(kt == 0),
                    stop=(kt == NKT - 1),
                )

        def norm_recip(h, py):
            # reciprocal of denominators: 1/d = exp(-ln d) on ScalarE (both
            # funcs live in the same activation table as the main exp stream);
            # emitted right after head h's pv completes to hide the latency
            lnd = small.tile([P, QCH], F32, tag="small", name="lnd")
            nc.scalar.activation(
                out=lnd[HEAD_DIM:HD1, :],
                in_=py[HEAD_DIM:HD1, :],
                func=AFT.Ln,
                bias=zbias[HEAD_DIM:HD1, :],
            )
            nc.scalar.activation(
                out=rec[h][HEAD_DIM:HD1, :],
                in_=lnd[HEAD_DIM:HD1, :],
                func=AFT.Exp,
                scale=-1.0,
                bias=zbias[HEAD_DIM:HD1, :],
            )

        def norm(pys):
            pbc = psB.tile([P, QCH], F32, tag="psB", name="pbc")
            for h in range(2):
                nc.tensor.matmul(
                    pbc[:], lhsT=fmat[h][:], rhs=rec[h][:],
                    start=(h == 0), stop=(h == 1),
                )
            rf = small.tile([P, QCH], F32, tag="small", name="rf")
            nc.vector.tensor_copy(out=rf[:], in_=pbc[:])
            yb = ybp.tile([P, QCH], BF16, tag="yb")
            nc.vector.tensor_mul(
                out=yb[0:HEAD_DIM, :], in0=pys[0][0:HEAD_DIM, :], in1=rf[0:HEAD_DIM, :]
            )
            nc.vector.tensor_mul(
                out=yb[HEAD_DIM:P, :], in0=pys[1][0:HEAD_DIM, :], in1=rf[HEAD_DIM:P, :]
            )
            return yb

        def proj_part(b, qc, yb, tt, cast_split=False):
            row0 = b * TB + qc * QCH + tt * P
            for ncol in range(C // QCH):
                po = psB.tile([P, QCH], F32, tag="psB", name="po")
                nc.tensor.matmul(
                    po[:],
                    lhsT=yb[:, tt * P : (tt + 1) * P],
                    rhs=wp_sb[:, ncol * QCH : (ncol + 1) * QCH],
                    start=True,
                    stop=True,
                )
                osb = opool.tile([P, QCH], BF16, tag="osb")
                if cast_split and ncol == 1:
                    # drain tail only: ScalarE is idle after its last exp, so
                    # splitting the psum->sbuf casts across both engines
                    # halves the per-tile proj chain
                    nc.scalar.copy(out=osb[:], in_=po[:])
                else:
                    nc.vector.tensor_copy(out=osb[:], in_=po[:])
                nc.sync.dma_start(
                    out=out[row0 : row0 + P, ncol * QCH : (ncol + 1) * QCH],
                    in_=osb[:],
                )

        chunks = [(b, qc) for b in range(NB) for qc in range(NQC)]
        pTs = {}

        def new_pT():
            return ppool.tile([P, NKT, 2, QCH], BF16, tag="pT", name="pT")

        # -------- phase A: batch-0 qkv with qc0 scores trickled in --------
        # chunk-0 x triggers split across the sync and (otherwise idle)
        # gpsimd DGE rings; constant memsets follow the DMA triggers so they
        # don't delay the gpsimd-issued loads
        xts = xt_load(0, engines=[nc.sync, nc.gpsimd])
        # wproj feeds only the (late) output projection: issue after the
        # prologue-critical chunk-0 x tiles
        for h in range(2):
            nc.sync.dma_start(
                out=wp_sb[h * HEAD_DIM : (h + 1) * HEAD_DIM, :], in_=wproj[h]
            )
        make_identity(nc, ident[:])
        # fmat[h] broadcasts the recip denominator (row 64) to that head's
        # 64-row block of the stacked y tile
        fmat = []
        for h in range(2):
            t = singles.tile([P, P], BF16, tag=f"fmat{h}", name=f"fmat{h}")
            nc.gpsimd.memset(t[:], 0.0)
            nc.gpsimd.memset(
                t[HEAD_DIM:HD1, h * HEAD_DIM : (h + 1) * HEAD_DIM], 1.0
            )
            fmat.append(t)
        rec = []
        for h in range(2):
            t = singles.tile([P, QCH], BF16, tag=f"rec{h}", name=f"rec{h}")
            nc.gpsimd.memset(t[:], 0.0)
            rec.append(t)
        zbias = singles.tile([P, 1], F32, tag="zbias", name="zbias")
        nc.gpsimd.memset(zbias[:], 0.0)
        qkv_chain(0, xts, 1, k_both)
        qkv_chain(0, xts, 0, q_sb)
        pTs[0] = new_pT()
        for kt in range(0, 4):
            sc_kt(0, 0, kt, pTs[0])
        vt = qkv_v_chain(0, xts)
        v_transpose(0, vt)
        for c in (1, 2, 3):
            xts = xt_load(c)
            qkv_chain(c, xts, 1, k_both)
            for kt in range(4 * c, 4 * c + 4):
                sc_kt(0, 0, kt, pTs[0])
            qkv_chain(c, xts, 0, q_sb)
            vt = qkv_v_chain(c, xts)
            v_transpose(c, vt)
        prefetched = {4: xt_load(4)}

        # -------- main slots: scores(j+1) interleaved at fine grain with
        # yproj(j) / batch-1 qkv so the PE queue never blocks on the exp
        # stream's psA WAR dependency --------
        deferred = []  # (b, qc, yb, next_tt) proj work pushed to exp-bound slots

        def drain_deferred(n):
            for _ in range(n):
                if deferred:
                    db, dqc, dyb, dtt = deferred[0]
                    proj_part(db, dqc, dyb, dtt)
                    if dtt == 3:
                        deferred.pop(0)
                    else:
                        deferred[0] = (db, dqc, dyb, dtt + 1)

        for j in range(NCH):
            b, qc = chunks[j]
            nxt = j + 1 if j + 1 < NCH else None
            bc = 4 + j if j < 4 else None  # batch-1 qkv handled this slot
            if bc is not None:
                qkv_chain(bc, prefetched[bc], 1, k_both)
            if nxt is not None:
                pTs[nxt] = new_pT()
                nb_, nqc_ = chunks[nxt]

            def sc_group(g):
                if nxt is not None:
                    for kt in (2 * g, 2 * g + 1):
                        sc_kt(nb_, nqc_, kt, pTs[nxt])

            if j == NCH - 1:
                # pv/norm for the last chunk were hoisted into the previous
                # slot: only its projection remains
                for tt in range(4):
                    proj_part(b, qc, yb_last, tt, cast_split=True)
                break
            pys = []
            for h in range(2):
                py = pyP.tile([P, QCH], F32, tag="pyP", name="py")
                pys.append(py)
            sc_group(0)
            pv_part(pys[0], b, 0, pTs[j], 0, 8)
            sc_group(1)
            pv_part(pys[0], b, 0, pTs[j], 8, 16)
            norm_recip(0, pys[0])
            sc_group(2)
            pv_part(pys[1], b, 1, pTs[j], 0, 8)
            sc_group(3)
            pv_part(pys[1], b, 1, pTs[j], 8, 16)
            norm_recip(1, pys[1])
            sc_group(4)
            if bc is not None:
                qkv_chain(bc, prefetched[bc], 0, q_sb)
            else:
                drain_deferred(3)
            sc_group(5)
            yb = norm(pys)
            if bc is not None:
                bvt = qkv_v_chain(bc, prefetched[bc])
            else:
                drain_deferred(2)
            sc_group(6)
            if bc is not None:
                v_transpose(bc, bvt)
                prefetched.pop(bc)
            else:
                drain_deferred(2)
            sc_group(7)
            if j < 4:
                deferred.append((b, qc, yb, 0))
                if bc is not None and bc + 1 < NCH:
                    prefetched[bc + 1] = xt_load(bc + 1)
            else:
                for tt in range(4):
                    proj_part(b, qc, yb, tt)
            if j == NCH - 2:
                # hoist the last chunk's pv + norm into this slot so the
                # drain tail is just its projection
                nb2, nqc2 = chunks[j + 1]
                pys2 = []
                for h in range(2):
                    py = pyP.tile([P, QCH], F32, tag="pyP", name="py")
                    pys2.append(py)
                pv_part(pys2[0], nb2, 0, pTs[j + 1], 0, 16)
                norm_recip(0, pys2[0])
                pv_part(pys2[1], nb2, 1, pTs[j + 1], 0, 16)
                norm_recip(1, pys2[1])
                yb_last = norm(pys2)
        while deferred:
            db, dqc, dyb, dtt = deferred.pop(0)
            for tt in range(dtt, 4):
                proj_part(db, dqc, dyb, tt)
    return nc


def _prepare_in_maps(x, w_attn, w_proj):
    bf16 = ml_dtypes.bfloat16
    x = np.asarray(x, dtype=np.float32)
    w_attn = np.asarray(w_attn, dtype=np.float32)
    w_proj = np.asarray(w_proj, dtype=np.float32)

    xT = np.ascontiguousarray(x.reshape(NT, C).T.astype(bf16))  # [C, NT]
    in_maps = []
    for c in range(N_CORES):
        h0, h1 = 2 * c, 2 * c + 1
        cols = []
        for h in (h0, h1):  # q columns, pre-scaled by softmax 1/sqrt(64)
            cols.append(w_attn[:, h * HEAD_DIM : (h + 1) * HEAD_DIM] * 0.125)
        for h in (h0, h1):  # k columns
            cols.append(w_attn[:, C + h * HEAD_DIM : C + (h + 1) * HEAD_DIM])
        for h in (h0, h1):  # v columns
            cols.append(w_attn[:, 2 * C + h * HEAD_DIM : 2 * C + (h + 1) * HEAD_DIM])
        wcat = np.concatenate(cols, axis=1).astype(bf16)  # [C, 384]
        wqkv_c = np.ascontiguousarray(wcat.reshape(KC, P, 384).transpose(1, 0, 2))
        wproj_c = np.ascontiguousarray(
            np.stack(
                [
                    w_proj[h0 * HEAD_DIM : (h0 + 1) * HEAD_DIM, :],
                    w_proj[h1 * HEAD_DIM : (h1 + 1) * HEAD_DIM, :],
                ]
            ).astype(bf16)
        )  # [2, 64, C]
        in_maps.append({"xT": xT, "wqkv": wqkv_c, "wproj": wproj_c})
    return in_maps


class _AttnBacc(bacc.Bacc):
    """Pin all activations to natural_log_exp_and_others so the per-head
    Ln/Exp reciprocal ops don't thrash ACT table loads against the big
    Exp ops."""

    def insert_act_table_loads(self):
        import bass_rust as _bass_rust
        from concourse.hw_specs import get_activation_tables

        has_activation = any(
            isinstance(i, mybir.InstActivation)
            for b in self.main_func.blocks
            for i in b.instructions
        )
        if not has_activation:
            return
        tables = []
        for name, fns in get_activation_tables(self.m.arch).items():
            if name != "natural_log_exp_and_others":
                fns = set()
            tables.append((name, fns))
        _bass_rust.insert_act_table_loads(self, tables)


_CACHED_NC = None


def _get_nc():
    global _CACHED_NC
    if _CACHED_NC is None:
        _CACHED_NC = _build_program(_AttnBacc())
        _CACHED_NC.finalize()
    return _CACHED_NC


def run(x, w_attn, w_proj, trace=False):
    """Returns (output [B, TB, C] float32, BassKernelResults)."""
    in_maps = _prepare_in_maps(x, w_attn, w_proj)
    nc = _get_nc()
    res = run_bass_kernel_spmd(nc, in_maps, core_ids=list(range(N_CORES)), trace=trace)
    acc = np.zeros((NT, C), dtype=np.float32)
    for r in res.results:
        acc += r["out"].astype(np.float32)
    return acc.reshape(NB, TB, C), res


def kernel(x, w_attn, w_proj):
    out, _ = run(x, w_attn, w_proj, trace=False)
    return out



# revision 62
# speedup vs baseline: 1.0110x; 1.0110x over previous
"""Multi-head attention forward (B=2, T=2048, C=1024, 16 heads of dim 64)
sharded 8-way tensor-parallel over heads across 8 TRN2 NeuronCores.

Each core computes 2 heads end-to-end:
  qkv^T = w_c^T @ x^T           (weight-stationary, produces transposed layout)
  S^T_h = k_h @ q_h^T           (head-dim contraction; the two heads' K=64
                                 matmuls use disjoint partition ranges so the
                                 PE packs them into one full-rate pass)
  P^T_h = exp(S^T_h)            (no max subtraction: scores are ~N(0,1), |S|<9)
  y^T_h = [v_h | 1]^T @ P^T_h   (ones column yields softmax denominators)
  out_c = sum_h (y_h/denom) @ w_proj[head rows]   (partial projection, bf16)
Host gathers: out = sum_c out_c  (the tensor-parallel all-reduce).

Pipeline layout: ScalarE's exp stream (the second-busiest resource after the
PE) starts as soon as chunk 0 of batch 0 is projected; in each main slot the
next chunk's score production is interleaved at 2-ktile grain with this
chunk's pv / batch-1 qkv / deferred projections so the in-order PE queue is
never parked behind the exp stream's PSUM WAR dependency. Per-head Ln/Exp
reciprocals are emitted immediately after each head's pv to hide their
latency, and the last chunk's pv+norm are hoisted into the prior slot so the
drain tail is just its projection.
"""

import numpy as np
import ml_dtypes
from contextlib import ExitStack

import concourse.bass as bass
import concourse.bacc as bacc
import concourse.mybir as mybir
import concourse.tile as tile
from concourse.bass_utils import run_bass_kernel_spmd
from concourse.masks import make_identity

F32 = mybir.dt.float32
BF16 = mybir.dt.bfloat16
AFT = mybir.ActivationFunctionType

P = 128
NB = 2        # batches
TB = 2048     # tokens per batch
NT = NB * TB  # 4096 tokens total
C = 1024
KC = C // P   # 8 contraction tiles for the qkv projection
QCH = 512     # q-token chunk
NQC = TB // QCH   # 4 q chunks per batch
NKT = TB // P     # 16 k tiles per batch
NCH = NB * NQC    # 8 chunks total
N_CORES = 8
HEAD_DIM = 64
HD1 = HEAD_DIM + 1


def _build_program(nc: bass.Bass):
    xT = nc.declare_dram_parameter("xT", [C, NT], BF16, isOutput=False)[:]
    # wqkv arrives pre-packed partition-major so its single DMA is 128
    # contiguous 6KB descriptors instead of 1024 x 768B
    wqkv = nc.declare_dram_parameter("wqkv", [P, KC, 384], BF16, isOutput=False)[:]
    wproj = nc.declare_dram_parameter("wproj", [2, HEAD_DIM, C], BF16, isOutput=False)[:]
    out = nc.declare_dram_parameter("out", [NT, C], BF16, isOutput=True)[:]

    with tile.TileContext(nc) as tc, ExitStack() as ctx:
        singles = ctx.enter_context(tc.tile_pool(name="singles", bufs=1))
        xin = ctx.enter_context(tc.tile_pool(name="xin", bufs=16))
        vtp = ctx.enter_context(tc.tile_pool(name="vtp", bufs=2))
        ppool = ctx.enter_context(tc.tile_pool(name="ppool", bufs=3))
        small = ctx.enter_context(tc.tile_pool(name="small", bufs=3))
        ybp = ctx.enter_context(tc.tile_pool(name="ybp", bufs=6))
        opool = ctx.enter_context(tc.tile_pool(name="opool", bufs=6))
        psA = ctx.enter_context(tc.tile_pool(name="psA", bufs=2, space="PSUM"))
        pyP = ctx.enter_context(tc.tile_pool(name="pyP", bufs=2, space="PSUM"))
        psB = ctx.enter_context(tc.tile_pool(name="psB", bufs=2, space="PSUM"))

        # ---------------- constants / persistent tensors ----------------
        # k columns (m=1) first: the first qkv chain is the k-chain, so it
        # can start as soon as this slice + the first x tiles land
        w_sb = singles.tile([P, KC, 384], BF16, tag="w_sb")
        for m in (1, 0, 2):
            nc.sync.dma_start(
                out=w_sb[:, :, m * P : (m + 1) * P],
                in_=wqkv[:, :, m * P : (m + 1) * P],
            )

        wp_sb = singles.tile([P, C], BF16, tag="wp")

        ident = singles.tile([P, P], BF16, tag="ident")
        make_identity(nc, ident[:])

        # fmat[h] broadcasts the recip denominator (row 64) to that head's
        # 64-row block of the stacked y tile
        fmat = []
        for h in range(2):
            t = singles.tile([P, P], BF16, tag=f"fmat{h}", name=f"fmat{h}")
            nc.gpsimd.memset(t[:], 0.0)
            nc.gpsimd.memset(
                t[HEAD_DIM:HD1, h * HEAD_DIM : (h + 1) * HEAD_DIM], 1.0
            )
            fmat.append(t)
        rec = []
        for h in range(2):
            t = singles.tile([P, QCH], BF16, tag=f"rec{h}", name=f"rec{h}")
            nc.gpsimd.memset(t[:], 0.0)
            rec.append(t)
        zbias = singles.tile([P, 1], F32, tag="zbias", name="zbias")
        nc.gpsimd.memset(zbias[:], 0.0)

        q_sb = singles.tile([P, NT], BF16, tag="q_sb")
        # k_both rows 0:64 = head0 k dims, rows 64:128 = head1 k dims
        k_both = singles.tile([P, NT], BF16, tag="k_both")
        # v_aug[:, i, h, :] = [v_h for token tile i (64 cols) | ones col]
        v_aug = singles.tile([P, NT // P, 2, HD1], BF16, tag="v_aug")
        nc.vector.memset(v_aug[:, :, :, HEAD_DIM:HD1], 1.0)

        # ---------------- building blocks ----------------
        def xt_load(t, engines=None):
            # engines: spread the DMA triggers across idle engine DGE rings
            # (prologue only) instead of serializing on the sync queue
            tsl = slice(t * QCH, (t + 1) * QCH)
            tiles = []
            for kc in range(KC):
                xt = xin.tile([P, QCH], BF16, tag="xin", name="xt")
                eng = engines[kc % len(engines)] if engines else nc.sync
                eng.dma_start(out=xt[:], in_=xT[kc * P : (kc + 1) * P, tsl])
                tiles.append(xt)
            return tiles

        def qkv_chain(t, xts, m, dest):
            # one 128-row slice (m=0: q both heads, m=1: k both heads)
            tsl = slice(t * QCH, (t + 1) * QCH)
            ps = psB.tile([P, QCH], F32, tag="psB", name="ps")
            for kc in range(KC):
                nc.tensor.matmul(
                    ps[:],
                    lhsT=w_sb[:, kc, m * P : (m + 1) * P],
                    rhs=xts[kc][:],
                    start=(kc == 0),
                    stop=(kc == KC - 1),
                )
            nc.vector.tensor_copy(out=dest[:, tsl], in_=ps[:])

        def qkv_v_chain(t, xts):
            ps = psB.tile([P, QCH], F32, tag="psB", name="ps")
            for kc in range(KC):
                nc.tensor.matmul(
                    ps[:],
                    lhsT=w_sb[:, kc, 2 * P : 3 * P],
                    rhs=xts[kc][:],
                    start=(kc == 0),
                    stop=(kc == KC - 1),
                )
            vt = vtp.tile([P, QCH], BF16, tag="vt")
            nc.vector.tensor_copy(out=vt[:], in_=ps[:])
            return vt

        def v_transpose(t, vt):
            pt = psB.tile([P, QCH], BF16, tag="psB", name="pt")
            for j in range(QCH // P):
                nc.tensor.transpose(
                    pt[:, j * P : (j + 1) * P], vt[:, j * P : (j + 1) * P], ident[:]
                )
            for j in range(QCH // P):
                i = t * (QCH // P) + j
                nc.vector.tensor_copy(
                    out=v_aug[:, i, 0, 0:HEAD_DIM], in_=pt[:, j * P : j * P + HEAD_DIM]
                )
                nc.vector.tensor_copy(
                    out=v_aug[:, i, 1, 0:HEAD_DIM],
                    in_=pt[:, j * P + HEAD_DIM : (j + 1) * P],
                )

        def sc_kt(b, qc, kt, pT):
            # both heads' scores in one pass: disjoint partition ranges let
            # the PE pack the two K=64 matmuls
            qsl = slice(b * TB + qc * QCH, b * TB + (qc + 1) * QCH)
            ksl = slice(b * TB + kt * P, b * TB + (kt + 1) * P)
            ps = psA.tile([P, 2 * QCH], F32, tag="psA", name="ps")
            nc.tensor.matmul(
                ps[:, 0:QCH],
                lhsT=k_both[0:HEAD_DIM, ksl],
                rhs=q_sb[0:HEAD_DIM, qsl],
                start=True,
                stop=True,
            )
            nc.tensor.matmul(
                ps[:, QCH : 2 * QCH],
                lhsT=k_both[HEAD_DIM:P, ksl],
                rhs=q_sb[HEAD_DIM:P, qsl],
                start=True,
                stop=True,
            )
            nc.scalar.activation(out=pT[:, kt, :, :], in_=ps[:], func=AFT.Exp)

        def pv_part(py, b, h, pT, k0, k1):
            for kt in range(k0, k1):
                nc.tensor.matmul(
                    py[0:HD1, :],
                    lhsT=v_aug[:, b * NKT + kt, h, :],
                    rhs=pT[:, kt, h, :],
                    start=(kt == 0),
                    stop=(kt == NKT - 1),
                )

        def norm_recip(h, py):
            # reciprocal of denominators: 1/d = exp(-ln d) on ScalarE (both
            # funcs live in the same activation table as the main exp stream);
            # emitted right after head h's pv completes to hide the latency
            lnd = small.tile([P, QCH], F32, tag="small", name="lnd")
            nc.scalar.activation(
                out=lnd[HEAD_DIM:HD1, :],
                in_=py[HEAD_DIM:HD1, :],
                func=AFT.Ln,
                bias=zbias[HEAD_DIM:HD1, :],
            )
            nc.scalar.activation(
                out=rec[h][HEAD_DIM:HD1, :],
                in_=lnd[HEAD_DIM:HD1, :],
                func=AFT.Exp,
                scale=-1.0,
                bias=zbias[HEAD_DIM:HD1, :],
            )

        def norm(pys):
            pbc = psB.tile([P, QCH], F32, tag="psB", name="pbc")
            for h in range(2):
                nc.tensor.matmul(
                    pbc[:], lhsT=fmat[h][:], rhs=rec[h][:],
                    start=(h == 0), stop=(h == 1),
                )
            rf = small.tile([P, QCH], F32, tag="small", name="rf")
            nc.vector.tensor_copy(out=rf[:], in_=pbc[:])
            yb = ybp.tile([P, QCH], BF16, tag="yb")
            nc.vector.tensor_mul(
                out=yb[0:HEAD_DIM, :], in0=pys[0][0:HEAD_DIM, :], in1=rf[0:HEAD_DIM, :]
            )
            nc.vector.tensor_mul(
                out=yb[HEAD_DIM:P, :], in0=pys[1][0:HEAD_DIM, :], in1=rf[HEAD_DIM:P, :]
            )
            return yb

        def proj_part(b, qc, yb, tt, cast_split=False):
            row0 = b * TB + qc * QCH + tt * P
            for ncol in range(C // QCH):
                po = psB.tile([P, QCH], F32, tag="psB", name="po")
                nc.tensor.matmul(
                    po[:],
                    lhsT=yb[:, tt * P : (tt + 1) * P],
                    rhs=wp_sb[:, ncol * QCH : (ncol + 1) * QCH],
                    start=True,
                    stop=True,
                )
                osb = opool.tile([P, QCH], BF16, tag="osb")
                if cast_split and ncol == 1:
                    # drain tail only: ScalarE is idle after its last exp, so
                    # splitting the psum->sbuf casts across both engines
                    # halves the per-tile proj chain
                    nc.scalar.copy(out=osb[:], in_=po[:])
                else:
                    nc.vector.tensor_copy(out=osb[:], in_=po[:])
                nc.sync.dma_start(
                    out=out[row0 : row0 + P, ncol * QCH : (ncol + 1) * QCH],
                    in_=osb[:],
                )

        chunks = [(b, qc) for b in range(NB) for qc in range(NQC)]
        pTs = {}

        def new_pT():
            return ppool.tile([P, NKT, 2, QCH], BF16, tag="pT", name="pT")

        # -------- phase A: batch-0 qkv with qc0 scores trickled in --------
        xts = xt_load(0)
        # wproj feeds only the (late) output projection: issue after the
        # prologue-critical chunk-0 x tiles
        for h in range(2):
            nc.sync.dma_start(
                out=wp_sb[h * HEAD_DIM : (h + 1) * HEAD_DIM, :], in_=wproj[h]
            )
        qkv_chain(0, xts, 1, k_both)
        qkv_chain(0, xts, 0, q_sb)
        pTs[0] = new_pT()
        for kt in range(0, 4):
            sc_kt(0, 0, kt, pTs[0])
        vt = qkv_v_chain(0, xts)
        v_transpose(0, vt)
        for c in (1, 2, 3):
            xts = xt_load(c)
            qkv_chain(c, xts, 1, k_both)
            for kt in range(4 * c, 4 * c + 4):
                sc_kt(0, 0, kt, pTs[0])
            qkv_chain(c, xts, 0, q_sb)
            vt = qkv_v_chain(c, xts)
            v_transpose(c, vt)
        prefetched = {4: xt_load(4)}

        # -------- main slots: scores(j+1) interleaved at fine grain with
        # yproj(j) / batch-1 qkv so the PE queue never blocks on the exp
        # stream's psA WAR dependency --------
        deferred = []  # (b, qc, yb, next_tt) proj work pushed to exp-bound slots

        def drain_deferred(n):
            for _ in range(n):
                if deferred:
                    db, dqc, dyb, dtt = deferred[0]
                    proj_part(db, dqc, dyb, dtt)
                    if dtt == 3:
                        deferred.pop(0)
                    else:
                        deferred[0] = (db, dqc, dyb, dtt + 1)

        for j in range(NCH):
            b, qc = chunks[j]
            nxt = j + 1 if j + 1 < NCH else None
            bc = 4 + j if j < 4 else None  # batch-1 qkv handled this slot
            if bc is not None:
                qkv_chain(bc, prefetched[bc], 1, k_both)
            if nxt is not None:
                pTs[nxt] = new_pT()
                nb_, nqc_ = chunks[nxt]

            def sc_group(g):
                if nxt is not None:
                    for kt in (2 * g, 2 * g + 1):
                        sc_kt(nb_, nqc_, kt, pTs[nxt])

            if j == NCH - 1:
                # pv/norm for the last chunk were hoisted into the previous
                # slot: only its projection remains
                for tt in range(4):
                    proj_part(b, qc, yb_last, tt, cast_split=True)
                break
            pys = []
            for h in range(2):
                py = pyP.tile([P, QCH], F32, tag="pyP", name="py")
                pys.append(py)
            sc_group(0)
            pv_part(pys[0], b, 0, pTs[j], 0, 8)
            sc_group(1)
            pv_part(pys[0], b, 0, pTs[j], 8, 16)
            norm_recip(0, pys[0])
            sc_group(2)
            pv_part(pys[1], b, 1, pTs[j], 0, 8)
            sc_group(3)
            pv_part(pys[1], b, 1, pTs[j], 8, 16)
            norm_recip(1, pys[1])
            sc_group(4)
            if bc is not None:
                qkv_chain(bc, prefetched[bc], 0, q_sb)
            else:
                drain_deferred(3)
            sc_group(5)
            yb = norm(pys)
            if bc is not None:
                bvt = qkv_v_chain(bc, prefetched[bc])
            else:
                drain_deferred(2)
            sc_group(6)
            if bc is not None:
                v_transpose(bc, bvt)
                prefetched.pop(bc)
            else:
                drain_deferred(2)
            sc_group(7)
            if j < 4:
                deferred.append((b, qc, yb, 0))
                if bc is not None and bc + 1 < NCH:
                    prefetched[bc + 1] = xt_load(bc + 1)
            else:
                for tt in range(4):
                    proj_part(b, qc, yb, tt)
            if j == NCH - 2:
                # hoist the last chunk's pv + norm into this slot so the
                # drain tail is just its projection
                nb2, nqc2 = chunks[j + 1]
                pys2 = []
                for h in range(2):
                    py = pyP.tile([P, QCH], F32, tag="pyP", name="py")
                    pys2.append(py)
                pv_part(pys2[0], nb2, 0, pTs[j + 1], 0, 16)
                norm_recip(0, pys2[0])
                pv_part(pys2[1], nb2, 1, pTs[j + 1], 0, 16)
                norm_recip(1, pys2[1])
                yb_last = norm(pys2)
        while deferred:
            db, dqc, dyb, dtt = deferred.pop(0)
            for tt in range(dtt, 4):
                proj_part(db, dqc, dyb, tt)
    return nc


def _prepare_in_maps(x, w_attn, w_proj):
    bf16 = ml_dtypes.bfloat16
    x = np.asarray(x, dtype=np.float32)
    w_attn = np.asarray(w_attn, dtype=np.float32)
    w_proj = np.asarray(w_proj, dtype=np.float32)

    xT = np.ascontiguousarray(x.reshape(NT, C).T.astype(bf16))  # [C, NT]
    in_maps = []
    for c in range(N_CORES):
        h0, h1 = 2 * c, 2 * c + 1
        cols = []
        for h in (h0, h1):  # q columns, pre-scaled by softmax 1/sqrt(64)
            cols.append(w_attn[:, h * HEAD_DIM : (h + 1) * HEAD_DIM] * 0.125)
        for h in (h0, h1):  # k columns
            cols.append(w_attn[:, C + h * HEAD_DIM : C + (h + 1) * HEAD_DIM])
        for h in (h0, h1):  # v columns
            cols.append(w_attn[:, 2 * C + h * HEAD_DIM : 2 * C + (h + 1) * HEAD_DIM])
        wcat = np.concatenate(cols, axis=1).astype(bf16)  # [C, 384]
        wqkv_c = np.ascontiguousarray(wcat.reshape(KC, P, 384).transpose(1, 0, 2))
        wproj_c = np.ascontiguousarray(
            np.stack(
                [
                    w_proj[h0 * HEAD_DIM : (h0 + 1) * HEAD_DIM, :],
                    w_proj[h1 * HEAD_DIM : (h1 + 1) * HEAD_DIM, :],
                ]
            ).astype(bf16)
        )  # [2, 64, C]
        in_maps.append({"xT": xT, "wqkv": wqkv_c, "wproj": wproj_c})
    return in_maps


class _AttnBacc(bacc.Bacc):
    """Pin all activations to natural_log_exp_and_others so the per-head
    Ln/Exp reciprocal ops don't thrash ACT table loads against the big
    Exp ops."""

    def insert_act_table_loads(self):
        import bass_rust as _bass_rust
        from concourse.hw_specs import get_activation_tables

        has_activation = any(
            isinstance(i, mybir.InstActivation)
            for b in self.main_func.blocks
            for i in b.instructions
        )
        if not has_activation:
            return
        tables = []
        for name, fns in get_activation_tables(self.m.arch).items():
            if name != "natural_log_exp_and_others":
                fns = set()
            tables.append((name, fns))
        _bass_rust.insert_act_table_loads(self, tables)


_CACHED_NC = None


def _get_nc():
    global _CACHED_NC
    if _CACHED_NC is None:
        _CACHED_NC = _build_program(_AttnBacc())
        _CACHED_NC.finalize()
    return _CACHED_NC


def run(x, w_attn, w_proj, trace=False):
    """Returns (output [B, TB, C] float32, BassKernelResults)."""
    in_maps = _prepare_in_maps(x, w_attn, w_proj)
    nc = _get_nc()
    res = run_bass_kernel_spmd(nc, in_maps, core_ids=list(range(N_CORES)), trace=trace)
    acc = np.zeros((NT, C), dtype=np.float32)
    for r in res.results:
        acc += r["out"].astype(np.float32)
    return acc.reshape(NB, TB, C), res


def kernel(x, w_attn, w_proj):
    out, _ = run(x, w_attn, w_proj, trace=False)
    return out

